# revision 1
# baseline (speedup 1.0000x reference)
"""Distributed Trainium2 kernel for nn_Attention_29832842838194.

LayerNorm (stats over the *sequence* axis) -> QKV projection -> 8-head
attention (N=2048, d_head=64) -> output projection, on 8 NeuronCores.

Sharding (v2 — token-parallel QKV, head-parallel attention):
  - tokens (B*N = 4096) split 8 ways; core c owns tokens [c*512, (c+1)*512)
    (all of one batch), computes LN partial stats and the full QKV
    projection for its tokens (M=128 full-PE-width matmuls).
  - LN stats: one batch-group AllReduce (groups {0..3}, {4..7}) of the
    16KB packed (sum, sumsq) vector — first collective, triggered ~10us
    in so the inter-core launch skew is absorbed while weights stream.
  - qkv reshard: three 512KB AllToAlls (k, then q, then v), each
    triggered as soon as its 4 M-chunks of the projection finish, so
    transfer overlaps the remaining matmuls. After the A2A core c holds
    q,k,v for head c over all 4096 tokens.
  - attention: head-parallel; sim matmuls (K=64) packed 2-at-a-time into
    the 128x128 PE via row tiling (kT/qT duplicated to partitions
    64..127); exp on ScalarE from fp32 PSUM with 1/sqrt(d) folded in;
    PV accumulates [65, 512] (65th row = softmax denominator).
  - output reshard: one 266KB AllToAll per batch; batch 0's normalize +
    out-projection is emitted under batch 1's attention.

Numerics: all matmuls bf16 with fp32 PSUM accumulation (matches the
baseline's accuracy envelope).

The kernel is self-contained: shapes are hardcoded to the problem spec.
"""

import numpy as np

# -------- problem constants (hardcoded per spec) --------
B = 2
NSEQ = 2048  # sequence length per batch
DIM = 1024
HEADS = 8
DHEAD = 64
INNER = HEADS * DHEAD  # 512
EPS = 1e-5
NCORES = 8
P = 128

SCALE = DHEAD ** -0.5  # 0.125


def _cfg(nseq=NSEQ):
    """Derived dims. nseq can be shrunk for simulator tests."""
    T = B * nseq              # total tokens
    TLOC = T // NCORES        # tokens per core
    TB = nseq // NCORES       # tokens per core per batch (out-a2a shard)
    assert TLOC % P == 0 and nseq % 512 == 0
    return dict(
        nseq=nseq,
        T=T,
        TLOC=TLOC,
        TB=TB,
        XT=TLOC // P,         # x token-subtiles per core (4)
        DC=DIM // P,          # 8 dmodel chunks
        KC=INNER // P,        # 4 inner chunks
        MC=3 * INNER // P,    # 12 qkv output chunks
        IB=nseq // 512,       # 512-query i-blocks per batch (4)
        JC=nseq // P,         # 128-key j-chunks per batch (16)
    )


def build_body(tc, outs, ins, cfg, dbg=False):
    """Emit the per-core program. outs/ins are dicts of DRAM APs."""
    import concourse.mybir as mybir
    from concourse.masks import make_identity
    from concourse.tile import add_dep_helper

    dt = mybir.dt
    AF = mybir.ActivationFunctionType
    ALU = mybir.AluOpType
    nc = tc.nc

    T, TLOC, TB, XT = cfg["T"], cfg["TLOC"], cfg["TB"], cfg["XT"]
    DC, KC, MC, IB, JC = cfg["DC"], cfg["KC"], cfg["MC"], cfg["IB"], cfg["JC"]
    nseq = cfg["nseq"]
    NTOK = float(nseq)  # tokens per batch (LN normalizer)
    RG = [list(range(NCORES))]
    # batch groups: cores 0..3 hold batch 0's tokens, 4..7 batch 1's
    half = NCORES // 2
    RGB = [list(range(half)), list(range(half, NCORES))]
    DE = DHEAD + 1

    x = ins["x"]      # [TLOC, DIM] f32 (this core's token slice)
    g = ins["g"]      # [DIM] f32
    w = ins["w"]      # [DIM, 3*INNER] f32: columns [k | q | v], head-major
    wo = ins["wo"]    # [INNER, DIM] f32 (replicated)
    out = outs["out"]  # [2*TB, DIM] f32 (rows: b*TB + t)

    with (
        tc.tile_pool(name="persist", bufs=1) as pp,
        tc.tile_pool(name="work", bufs=3) as pool,
        tc.tile_pool(name="work2", bufs=2) as pool2,
        tc.tile_pool(name="pref", bufs=8) as pref,
        tc.tile_pool(name="psum", bufs=2, space="PSUM") as psum,
        tc.tile_pool(name="dram", bufs=1, space="DRAM") as dram,
    ):
        # -------- constants
        ident = pp.tile([P, P], dt.bfloat16)
        make_identity(nc, ident)
        g_sb = pp.tile([P, DC], dt.float32)
        nc.sync.dma_start(g_sb[:], g.rearrange("(c p) -> p c", p=P))

        # -------- phase 0: x load -> cast -> transpose; LN partial stats
        # pipelined per d-chunk straight out of PSUM so the group
        # AllReduce triggers ASAP (the first collective absorbs skew)
        x_bf = pp.tile([P, XT, DIM], dt.bfloat16)
        for t in range(XT):
            x_f = pool.tile([P, DIM], dt.float32, tag="xload")
            nc.sync.dma_start(x_f[:], x[t * P:(t + 1) * P, :])
            nc.vector.tensor_copy(x_bf[:, t, :], x_f[:])
        # xT[p, dc, t] = x[t, dc*128+p] (bf16)
        xT = pp.tile([P, DC, TLOC], dt.bfloat16)
        ar_sb = pp.tile([P, 2 * DC], dt.float32)
        for dc in range(DC):
            ps = psum.tile([P, TLOC], dt.bfloat16, tag="tr")
            for t in range(XT):
                nc.tensor.transpose(
                    ps[:, t * P:(t + 1) * P],
                    x_bf[:, t, dc * P:(dc + 1) * P],
                    ident[:],
                )
            nc.vector.tensor_reduce(
                ar_sb[:, dc:dc + 1], ps[:], axis=mybir.AxisListType.X,
                op=ALU.add,
            )
            trash = pool2.tile([P, TLOC], dt.float32, tag="trash")
            nc.scalar.activation(
                trash[:], ps[:], AF.Square,
                accum_out=ar_sb[:, DC + dc:DC + dc + 1],
            )
            nc.vector.tensor_copy(xT[:, dc, :], ps[:])
        ar_in = dram.tile([P, 2 * DC], dt.float32)
        ar_dma = nc.sync.dma_start(ar_in[:], ar_sb[:])
        ar_out = dram.tile([P, 2 * DC], dt.float32, tag="arout")
        nc.gpsimd.collective_compute(
            "AllReduce", ALU.add, replica_groups=RGB,
            ins=[ar_in.opt()], outs=[ar_out.opt()],
        )

        # -------- weights (stream + cast while the AllReduce flies).
        # Gate the 8.4MB of weight DMAs behind the tiny AR-input DMA so
        # they don't steal HBM bandwidth from the stats critical path.
        w_bf = pp.tile([P, DC, 3 * INNER], dt.bfloat16)
        for kc in range(DC):
            wl = pool.tile([P, 3 * INNER], dt.float32, tag="wload")
            wdma = nc.sync.dma_start(wl[:], w[kc * P:(kc + 1) * P, :])
            add_dep_helper(wdma.ins, ar_dma.ins,
                           reason="defer weight loads behind stats AR input")
            nc.vector.tensor_copy(w_bf[:, kc, :], wl[:])
        wo_bf = pp.tile([P, KC, DIM], dt.bfloat16)
        for kc in range(KC):
            wol = pool.tile([P, DIM], dt.float32, tag="wload")
            wdma = nc.sync.dma_start(wol[:], wo[kc * P:(kc + 1) * P, :])
            add_dep_helper(wdma.ins, ar_dma.ins,
                           reason="defer weight loads behind stats AR input")
            nc.vector.tensor_copy(wo_bf[:, kc, :], wol[:])

        # head-broadcast selector for the rownorm: sel[h, kc, m] = 1 iff
        # h == 2*kc + (m >= DHEAD)
        sel_np = np.zeros((NCORES, KC, P), np.float32)
        for kc in range(KC):
            sel_np[2 * kc, kc, 0:DHEAD] = 1.0
            sel_np[2 * kc + 1, kc, DHEAD:P] = 1.0
        sel_dram = nc.inline_tensor(sel_np, name="selmat")
        sel = pp.tile([NCORES, KC, P], dt.float32)
        nc.sync.dma_start(sel[:], sel_dram.ap())

        # -------- LN coefficients (this core's batch only)
        stats = pp.tile([P, 2 * DC], dt.float32)
        nc.sync.dma_start(stats[:], ar_out[:])
        mean = pp.tile([P, DC], dt.float32)
        nc.vector.tensor_scalar_mul(mean[:], stats[:, 0:DC], 1.0 / NTOK)
        e2 = pp.tile([P, DC], dt.float32)
        nc.vector.tensor_scalar_mul(e2[:], stats[:, DC:2 * DC], 1.0 / NTOK)
        msq = pp.tile([P, DC], dt.float32)
        nc.vector.tensor_tensor(msq[:], mean[:], mean[:], ALU.mult)
        vareps = pp.tile([P, DC], dt.float32)
        nc.vector.tensor_tensor(vareps[:], e2[:], msq[:], ALU.subtract)
        nc.vector.tensor_scalar_add(vareps[:], vareps[:], EPS)
        rvar = pp.tile([P, DC], dt.float32)
        nc.vector.reciprocal(rvar[:], vareps[:])
        rstd = pp.tile([P, DC], dt.float32)
        nc.scalar.activation(rstd[:], rvar[:], AF.Sqrt)
        A2 = pp.tile([P, DC], dt.float32)
        nc.vector.tensor_tensor(A2[:], rstd[:], g_sb[:], ALU.mult)
        C2 = pp.tile([P, DC], dt.float32)
        nc.vector.tensor_tensor(C2[:], mean[:], A2[:], ALU.mult)
        nc.vector.tensor_scalar_mul(C2[:], C2[:], -1.0)
        # preload the exp table while the PE chews on QKV
        junk = pp.tile([1, DC], dt.float32)
        nc.scalar.activation(junk[:], A2[0:1, :], AF.Exp)
        # warm the PE (HAM throttle) during the LN-coefficient window so
        # QKV runs at 2.4GHz: ~3.4us of junk matmuls gated on the AR
        stats_bf = pp.tile([P, 2 * DC], dt.bfloat16)
        nc.vector.tensor_copy(stats_bf[:], stats[:])
        for wu in range(8):
            jp = psum.tile([2 * DC, TLOC], dt.float32, tag="tr")
            nc.tensor.matmul(jp[:], stats_bf[:], xT[:, wu % DC, :],
                             start=True, stop=True)
        if dbg:
            nc.sync.dma_start(outs["dbg_stats"], stats[:])
            nc.sync.dma_start(outs["dbg_A2"], A2[:])
            nc.sync.dma_start(outs["dbg_C2"], C2[:])

        # normalize xT in place
        for dc in range(DC):
            nc.vector.tensor_scalar(
                xT[:, dc, :], xT[:, dc, :],
                A2[:, dc:dc + 1], C2[:, dc:dc + 1],
                ALU.mult, ALU.add,
            )

        # -------- QKV projection (12 M-chunks; w columns are [k | q | v]).
        # [k,q] AllToAll fires after chunk 7 so the ACT exp pipeline can
        # start ~12us earlier; v follows (serialized on the cc stream but
        # PV has slack until the first exps are done).
        kq_in = dram.tile([NCORES, 2, DHEAD, TLOC], dt.bfloat16, tag="kqi")
        v_in = dram.tile([NCORES, TLOC, DHEAD], dt.bfloat16, tag="vi")
        kq_out = dram.tile([NCORES, 2, DHEAD, TLOC], dt.bfloat16, tag="kqo")
        v_out = dram.tile([NCORES, TLOC, DHEAD], dt.bfloat16, tag="vo")
        for mc in range(2 * KC):
            qp = psum.tile([P, TLOC], dt.float32, tag="acc")
            for kc in range(DC):
                nc.tensor.matmul(
                    qp[:], w_bf[:, kc, mc * P:(mc + 1) * P], xT[:, kc, :],
                    start=(kc == 0), stop=(kc == DC - 1),
                )
            qsb = pool.tile([P, TLOC], dt.bfloat16, tag="qsb")
            nc.vector.tensor_copy(qsb[:], qp[:])
            grp, mg = divmod(mc, KC)
            for hh in range(2):
                nc.sync.dma_start(kq_in[2 * mg + hh, grp],
                                  qsb[hh * DHEAD:(hh + 1) * DHEAD, :])
        nc.gpsimd.collective_compute(
            "AllToAll", ALU.bypass, replica_groups=RG,
            ins=[kq_in.opt()], outs=[kq_out.opt()],
        )
        # v computed pre-transposed ([token, vcol]) on the sender, so the
        # receiver needs no PE transposes before PV can start
        for tc in range(XT):
            vp = psum.tile([P, INNER], dt.float32, tag="acc")
            for kc in range(DC):
                nc.tensor.matmul(
                    vp[:], xT[:, kc, tc * P:(tc + 1) * P],
                    w_bf[:, kc, 2 * INNER:3 * INNER],
                    start=(kc == 0), stop=(kc == DC - 1),
                )
            vsb = pool.tile([P, INNER], dt.bfloat16, tag="qsb")
            nc.vector.tensor_copy(vsb[:], vp[:])
            nc.sync.dma_start(
                v_in[:, tc * P:(tc + 1) * P, :].rearrange("h p d -> p h d"),
                vsb[:].rearrange("p (h d) -> p h d", h=NCORES),
            )
        nc.gpsimd.collective_compute(
            "AllToAll", ALU.bypass, replica_groups=RG,
            ins=[v_in.opt()], outs=[v_out.opt()],
        )

        # gather: kT2/qT2 [128, T] with rows 64..127 duplicating 0..63
        # (row-tiled sim runs two j-chunks concurrently); vT [65, T] with
        # the ones row for the softmax denominator.
        kT2 = pp.tile([P, T], dt.bfloat16)
        qT2 = pp.tile([P, T], dt.bfloat16)
        kq_r = kq_out.rearrange("r s d t -> s d r t")

        def _split(ap):
            return ap.rearrange("d (r t) -> d r t", r=NCORES)

        nc.sync.dma_start(_split(kT2[0:DHEAD, :]), kq_r[0])
        nc.sync.dma_start(_split(kT2[DHEAD:P, :]), kq_r[0])
        nc.sync.dma_start(_split(qT2[0:DHEAD, :]), kq_r[1])
        nc.sync.dma_start(_split(qT2[DHEAD:P, :]), kq_r[1])
        # vext[j, jc, d|1]: ones column preset, data straight off the A2A
        NJC = T // P  # 32 j-chunks over both batches
        vext = pp.tile([P, NJC, DE], dt.bfloat16)
        nc.gpsimd.memset(vext[:, :, DHEAD:DE], 1.0)
        nc.sync.dma_start(
            vext[:, :, 0:DHEAD],
            v_out.rearrange("r (jc p) d -> p (r jc) d", p=P),
        )
        if dbg:
            nc.sync.dma_start(outs["dbg_xn"], xT[:])
            nc.sync.dma_start(outs["dbg_kT"], kT2[:])
            nc.sync.dma_start(outs["dbg_qT"], qT2[:])
            nc.sync.dma_start(outs["dbg_vT"], vext[:])

        # -------- attention + output A2As, postprocess interleaved
        aoT = pp.tile([DE, T], dt.bfloat16)

        def attn_sims(b, ib, et_pool, et_tag):
            """Emit sims + exps only (for the window where v is in flight)."""
            i0 = b * nseq + ib * 512
            ets = []
            for jg in range(JC // 2):
                sp = psum.tile([P, 1024], dt.float32, tag="sim")
                for u in range(2):
                    j0 = b * nseq + (jg * 2 + u) * P
                    rsl = slice(u * DHEAD, u * DHEAD + DHEAD)
                    nc.tensor.matmul(
                        sp[:, u * 512:(u + 1) * 512],
                        kT2[rsl, j0:j0 + P], qT2[rsl, i0:i0 + 512],
                        start=True, stop=True,
                    )
                et = et_pool.tile([P, 1024], dt.bfloat16, tag=et_tag)
                nc.scalar.activation(et[:], sp[:], AF.Exp, scale=SCALE)
                ets.append(et)
            return ets

        def attn_pvs(b, ib, ets):
            i0 = b * nseq + ib * 512
            otp = psum.tile([DE, 512], dt.float32, tag="acc")
            for jg in range(JC // 2):
                for u in range(2):
                    jc = jg * 2 + u
                    nc.tensor.matmul(
                        otp[:], vext[:, b * JC + jc, :],
                        ets[jg][:, u * 512:(u + 1) * 512],
                        start=(jg == 0 and u == 0),
                        stop=(jg == JC // 2 - 1 and u == 1),
                    )
            nc.vector.tensor_copy(aoT[:, i0:i0 + 512], otp[:])

        def attn_block(b, ib):
            i0 = b * nseq + ib * 512
            otp = psum.tile([DE, 512], dt.float32, tag="acc")
            for jg in range(JC // 2):
                sp = psum.tile([P, 1024], dt.float32, tag="sim")
                for u in range(2):
                    j0 = b * nseq + (jg * 2 + u) * P
                    rsl = slice(u * DHEAD, u * DHEAD + DHEAD)
                    nc.tensor.matmul(
                        sp[:, u * 512:(u + 1) * 512],
                        kT2[rsl, j0:j0 + P], qT2[rsl, i0:i0 + 512],
                        start=True, stop=True,
                    )
                et = pool.tile([P, 1024], dt.bfloat16, tag="exp")
                nc.scalar.activation(et[:], sp[:], AF.Exp, scale=SCALE)
                for u in range(2):
                    jc = jg * 2 + u
                    nc.tensor.matmul(
                        otp[:], vext[:, b * JC + jc, :],
                        et[:, u * 512:(u + 1) * 512],
                        start=(jg == 0 and u == 0),
                        stop=(jg == JC // 2 - 1 and u == 1),
                    )
            nc.vector.tensor_copy(aoT[:, i0:i0 + 512], otp[:])

        def out_a2a(off, tb, tag):
            a2a_in = dram.tile([NCORES, DE, tb], dt.bfloat16,
                               tag=f"oa{tag}")
            nc.sync.dma_start(
                a2a_in.rearrange("s d t -> d s t"),
                aoT[:, off:off + NCORES * tb].rearrange(
                    "d (s t) -> d s t", s=NCORES),
            )
            a2a_out = dram.tile([NCORES, DE, tb], dt.bfloat16,
                                tag=f"ob{tag}")
            nc.gpsimd.collective_compute(
                "AllToAll", ALU.bypass, replica_groups=RG,
                ins=[a2a_in.opt()], outs=[a2a_out.opt()],
            )
            return a2a_out

        def pp_stages(row0, tb, a2a_out):
            """Postprocess split into stages so the PE work trickles into
            the exp-stream's slack instead of lumping into one stall."""
            st = {}

            def s_gather():
                ao_g = pool2.tile([P, KC, tb], dt.bfloat16, tag="aog")
                a2a_v = a2a_out.rearrange("(kc rr) d t -> rr d kc t", rr=2)
                for rr in range(2):
                    nc.scalar.dma_start(
                        ao_g[rr * DHEAD:(rr + 1) * DHEAD, :, :],
                        a2a_v[rr, 0:DHEAD],
                    )
                rn = pool2.tile([NCORES, tb], dt.bfloat16, tag="rn")
                nc.scalar.dma_start(rn[:], a2a_out[:, DHEAD, :])
                rc = pool2.tile([NCORES, tb], dt.float32, tag="rc")
                nc.vector.reciprocal(rc[:], rn[:])
                st["ao_g"], st["rc"] = ao_g, rc

            def s_norm():
                ao_g, rc = st["ao_g"], st["rc"]
                for kc in range(KC):
                    bcp = psum.tile([P, tb], dt.float32, tag="tr")
                    nc.tensor.matmul(bcp[:], sel[:, kc, :], rc[:],
                                     start=True, stop=True)
                    nc.vector.tensor_tensor(
                        ao_g[:, kc, :], ao_g[:, kc, :], bcp[:], ALU.mult
                    )

            def s_proj(t0):
                ao_g = st["ao_g"]
                mw = min(P, tb - t0)
                out_sb = pool2.tile([P, DIM], dt.float32, tag="osb")
                for nh2 in range(DIM // 512):
                    op = psum.tile([P, 512], dt.float32, tag="tr")
                    for kc in range(KC):
                        nc.tensor.matmul(
                            op[0:mw, :], ao_g[:, kc, t0:t0 + mw],
                            wo_bf[:, kc, nh2 * 512:(nh2 + 1) * 512],
                            start=(kc == 0), stop=(kc == KC - 1),
                        )
                    nc.vector.tensor_copy(
                        out_sb[0:mw, nh2 * 512:(nh2 + 1) * 512], op[0:mw, :]
                    )
                nc.scalar.dma_start(out[row0 + t0:row0 + t0 + mw, :],
                                    out_sb[0:mw, :])

            stages = [s_gather, s_norm]
            stages += [lambda t0=t0: s_proj(t0) for t0 in range(0, tb, P)]
            return stages

        TBH = TB // 2
        # first two i-blocks: sims+exps run while the v A2A is in flight,
        # PVs follow once vext lands
        ets0 = attn_sims(0, 0, pref, "pref0")
        ets1 = attn_sims(0, 1, pref, "pref1")
        attn_pvs(0, 0, ets0)
        attn_pvs(0, 1, ets1)
        attn_block(0, 2)
        attn_block(0, 3)
        o0 = out_a2a(0, TB, "b0")
        pp0 = pp_stages(0, TB, o0)
        attn_block(1, 0)
        pp0[0]()                        # gather+rc (no PE work)
        attn_block(1, 1)
        pp0[1]()                        # rownorm broadcast
        o10 = out_a2a(nseq, TBH, "b1h0")
        pp10 = pp_stages(TB, TBH, o10)
        attn_block(1, 2)
        pp0[2]()                        # b0 out-projection, first half
        attn_block(1, 3)
        o11 = out_a2a(nseq + NCORES * TBH, TBH, "b1h1")
        pp11 = pp_stages(TB + TBH, TBH, o11)
        pp0[3]()                        # b0 out-projection, second half
        for s in pp10:                  # b1 first half, under b1h1's A2A
            s()
        for s in pp11:
            s()
        if dbg:
            nc.sync.dma_start(outs["dbg_aoT"], aoT[:])


def build_graph(cfg, dbg=False):
    import concourse.mybir as mybir
    import concourse.tile as tile
    from concourse import bacc

    dt = mybir.dt
    nc = bacc.Bacc("TRN2", target_bir_lowering=False, debug=False,
                   num_devices=NCORES)
    TLOC, TB = cfg["TLOC"], cfg["TB"]
    T, DC = cfg["T"], cfg["DC"]
    ins = {
        "x": nc.dram_tensor("x", [TLOC, DIM], dt.float32, kind="ExternalInput").ap(),
        "g": nc.dram_tensor("g", [DIM], dt.float32, kind="ExternalInput").ap(),
        "w": nc.dram_tensor("w", [DIM, 3 * INNER], dt.float32, kind="ExternalInput").ap(),
        "wo": nc.dram_tensor("wo", [INNER, DIM], dt.float32, kind="ExternalInput").ap(),
    }
    outs = {
        "out": nc.dram_tensor("out", [B * TB, DIM], dt.float32,
                              kind="ExternalOutput").ap(),
    }
    if dbg:
        for name, shape, dt_ in (
            ("dbg_stats", [P, 2 * DC], dt.float32),
            ("dbg_A2", [P, DC], dt.float32),
            ("dbg_C2", [P, DC], dt.float32),
            ("dbg_xn", [P, DC, TLOC], dt.bfloat16),
            ("dbg_kT", [P, T], dt.bfloat16),
            ("dbg_qT", [P, T], dt.bfloat16),
            ("dbg_vT", [P, T // P, DHEAD + 1], dt.bfloat16),
            ("dbg_aoT", [DHEAD + 1, T], dt.bfloat16),
        ):
            outs[name] = nc.dram_tensor(name, shape, dt_,
                                        kind="ExternalOutput").ap()
    with tile.TileContext(nc) as tc:
        build_body(tc, outs, ins, cfg, dbg=dbg)
    nc.compile()
    return nc


def make_in_maps(x, g, wq, wkv, wo, cfg):
    """Shard full inputs into per-core input maps."""
    T, TLOC = cfg["T"], cfg["TLOC"]
    x2 = np.ascontiguousarray(np.asarray(x, np.float32).reshape(T, DIM))
    g_ = np.ascontiguousarray(np.asarray(g, np.float32))
    wq_ = np.asarray(wq, np.float32)
    wkv_ = np.asarray(wkv, np.float32)
    wo_ = np.ascontiguousarray(np.asarray(wo, np.float32))
    # columns [k | q | v], head-major inside each block
    w_cat = np.ascontiguousarray(
        np.concatenate([wkv_[:, :INNER], wq_, wkv_[:, INNER:]], axis=1)
    )
    in_maps = []
    for c in range(NCORES):
        in_maps.append({
            "x": np.ascontiguousarray(x2[c * TLOC:(c + 1) * TLOC]),
            "g": g_,
            "w": w_cat,
            "wo": wo_,
        })
    return in_maps


def assemble_out(core_outs, cfg):
    """Batch 0 resharded in TB-token shards, batch 1 in two TB/2 halves."""
    T, TB = cfg["T"], cfg["TB"]
    nseq = cfg["nseq"]
    TBH = TB // 2
    full = np.empty((T, DIM), np.float32)
    for c in range(NCORES):
        o = core_outs[c]
        full[c * TB:(c + 1) * TB] = o[0:TB]
        for h in range(2):
            src = TB + h * TBH
            dst = nseq + h * NCORES * TBH + c * TBH
            full[dst:dst + TBH] = o[src:src + TBH]
    return full


_cache = {}


def _get_graph():
    if "nc" not in _cache:
        _cache["nc"] = build_graph(_cfg())
    return _cache["nc"]


def run_on_hw(in_maps, trace=False, **kw):
    from concourse.bass_utils import run_bass_kernel_spmd
    nc = _get_graph()
    return run_bass_kernel_spmd(
        nc, in_maps, core_ids=list(range(NCORES)), trace=trace, **kw
    )


def kernel(x, g, wq, wkv, wo):
    cfg = _cfg()
    in_maps = make_in_maps(x, g, wq, wkv, wo, cfg)
    res = run_on_hw(in_maps)
    core_outs = [np.asarray(res.results[c]["out"], np.float32)
                 for c in range(NCORES)]
    return assemble_out(core_outs, cfg).reshape(B, NSEQ, DIM)

